# revision 20
# baseline (speedup 1.0000x reference)
"""Trainium2 Bass kernel: decode attention with a 32K KV cache.

Problem: x[32,1024] -> qkv proj (16 heads, dh=64) -> attention over
(32768 cached + 32 new) keys -> c_proj. Returns (out[32,1024],
present[2,16,32800,64]).

Sharding: 2 heads per core (tensor parallel over n_head=16).
Each core gets:
  kt    [128, 32768]  K^T for its 2 heads (d-major: rows 0:64 head A,
                      64:128 head B) -- stationary operand of the scores
                      matmuls, streamed in 4096-key chunks.
  vaug  [32768, 130]  V rows [v_h0(64) | 1 | v_h1(64) | 1]; the ones
                      columns make the PV matmul also produce the softmax
                      denominator. Rows are host-permuted so the per-chunk
                      DMA is 16.6KB-contiguous per partition.
  xT, wq/wk/wv (per-core columns of w_attn; wq pre-scaled by 1/8),
  bqkv, wp (per-core rows of w_proj), maskT (causal mask for the 32 new
  keys).
Each core returns its partial c_proj output po[32,1024] (contribution of
its 2 heads); the host sums the 8 partials and adds b_proj. No on-device
collective needed. knew/vnew [128,32] return the new k/v for `present`.

Device algorithm per head (S^T layout, no max-subtraction -- scores are
q.k/8 with |score| < ~2, exp is safe in fp32; masked new-block entries
are zeroed multiplicatively after exp):
  per 128-key group g: S^T[g] [128,32] = (K^T chunk).T @ qT   (PSUM)
  per 2048 keys: P^T = exp(S^T bank [128,512])                (ACT->SBUF)
  per group: out[32,65] += P^T[g].T @ [V|1][g]                (PSUM accum)
  epilogue: a = out[:, :64] / out[:, 64:65]; partial = a_merged @ wp.
"""

import numpy as np
from contextlib import ExitStack

import concourse.bass as bass
import concourse.tile as tile
from concourse import bacc, mybir
from concourse.bass_utils import run_bass_kernel_spmd
from concourse.masks import make_identity

N_EMBD = 1024
N_HEAD = 16
DH = 64
S = 32
PAST = 32768
NCORES = 8
HPC = N_HEAD // NCORES        # 2 heads per core
DPC = HPC * DH                # 128 dims per core
F32 = mybir.dt.float32
BF16 = mybir.dt.bfloat16
FP16 = mybir.dt.float16

# tuning knobs
CHUNK = 8192                  # keys per DMA chunk
SGK = 2048                    # keys per PSUM scores bank (16 groups x 128)
KV_BUFS = 3                   # double-buffer the kv chunk tiles
PT_BUFS = 4
K_LOW = True                  # low-precision K (and q) for scores
V_LOW = True                  # low-precision V (and P^T) for PV
QKV_LOW = True                # low-precision qkv-projection weights + xT
LOW_DT = "fp16"               # "fp16" | "bf16" 

_CACHE = {}


def _build():
    key = (CHUNK, SGK, KV_BUFS, PT_BUFS, K_LOW, V_LOW, QKV_LOW, LOW_DT)
    if key in _CACHE:
        return _CACHE[key]

    NCHUNK = PAST // CHUNK
    NSG = CHUNK // SGK        # supergroups per chunk
    GPS = SGK // 128          # 128-key groups per supergroup
    LDT = FP16 if LOW_DT == "fp16" else BF16
    KDT = LDT if K_LOW else F32
    VDT = LDT if V_LOW else F32
    WDT = LDT if QKV_LOW else F32

    nc = bacc.Bacc("TRN2", target_bir_lowering=False, debug=False,
                   enable_asserts=False, num_devices=NCORES)

    kt = nc.dram_tensor("kt", [DPC, PAST], KDT, kind="ExternalInput").ap()
    vaug = nc.dram_tensor("vaug", [PAST, 130], VDT, kind="ExternalInput").ap()
    xT = nc.dram_tensor("xT", [N_EMBD, S], WDT, kind="ExternalInput").ap()
    wq = nc.dram_tensor("wq", [N_EMBD, DPC], WDT, kind="ExternalInput").ap()
    wk = nc.dram_tensor("wk", [N_EMBD, DPC], WDT, kind="ExternalInput").ap()
    wv = nc.dram_tensor("wv", [N_EMBD, DPC], WDT, kind="ExternalInput").ap()
    bqkv = nc.dram_tensor("bqkv", [DPC, 3], F32, kind="ExternalInput").ap()
    wp = nc.dram_tensor("wp", [DPC, N_EMBD], F32, kind="ExternalInput").ap()
    maskT = nc.dram_tensor("maskT", [S, 2 * S], VDT, kind="ExternalInput").ap()
    po = nc.dram_tensor("po", [S, N_EMBD], F32, kind="ExternalOutput").ap()
    knew = nc.dram_tensor("knew", [DPC, S], F32, kind="ExternalOutput").ap()
    vnew = nc.dram_tensor("vnew", [DPC, S], F32, kind="ExternalOutput").ap()

    EXP = mybir.ActivationFunctionType.Exp

    with tile.TileContext(nc) as tc, ExitStack() as ctx:
        const = ctx.enter_context(tc.tile_pool(name="const", bufs=1))
        kpool = ctx.enter_context(tc.tile_pool(name="kpool", bufs=KV_BUFS))
        vpool = ctx.enter_context(tc.tile_pool(name="vpool", bufs=KV_BUFS))
        ppool = ctx.enter_context(tc.tile_pool(name="ppool", bufs=PT_BUFS))
        spsum = ctx.enter_context(tc.tile_pool(name="spsum", bufs=4, space="PSUM"))
        opsum = ctx.enter_context(tc.tile_pool(name="opsum", bufs=1, space="PSUM"))
        mpsum = ctx.enter_context(tc.tile_pool(name="mpsum", bufs=2, space="PSUM"))

        # ---- constants / weights ----
        xT_sb = const.tile([128, 8, S], WDT, name="xT_sb", tag="xT_sb")
        nc.gpsimd.dma_start(xT_sb, xT.rearrange("(c p) s -> p c s", p=128))
        w_sb = {}
        for nm, w in (("wq", wq), ("wk", wk), ("wv", wv)):
            t = const.tile([128, 8, DPC], WDT, name=f"{nm}_sb", tag=f"{nm}_sb")
            nc.gpsimd.dma_start(t, w.rearrange("(c p) f -> p c f", p=128))
            w_sb[nm] = t
        b_sb = const.tile([DPC, 3], F32, name="b_sb", tag="b_sb")
        nc.gpsimd.dma_start(b_sb, bqkv)
        mask_sb = const.tile([S, 2 * S], VDT, name="mask_sb", tag="mask_sb")
        nc.gpsimd.dma_start(mask_sb, maskT)
        wp_sb = const.tile([DPC, N_EMBD], F32, name="wp_sb", tag="wp_sb")
        nc.gpsimd.dma_start(wp_sb, wp)
        ident = const.tile([128, 128], F32, name="ident", tag="ident")
        make_identity(nc, ident)

        # ---- qkv projection (transposed layout [f, s]) ----
        qkv = {}
        for bi, nm in enumerate(("wq", "wk", "wv")):
            ps = mpsum.tile([128, S], F32, name=f"qkvp_{nm}", tag="mp")
            for c in range(8):
                nc.tensor.matmul(ps, lhsT=w_sb[nm][:, c, :], rhs=xT_sb[:, c, :],
                                 start=(c == 0), stop=(c == 7))
            sb = const.tile([128, S], F32, name=f"{nm}t", tag=f"{nm}t")
            nc.vector.tensor_scalar_add(sb, ps, b_sb[:, bi:bi + 1])
            qkv[nm] = sb
        q_sb, k_sb, v_sb = qkv["wq"], qkv["wk"], qkv["wv"]

        nc.gpsimd.dma_start(knew, k_sb)
        nc.gpsimd.dma_start(vnew, v_sb)

        if K_LOW:
            q_mm = const.tile([128, S], LDT, name="q_lo", tag="q_lo")
            nc.vector.tensor_copy(out=q_mm, in_=q_sb)
            k_mm = const.tile([128, S], LDT, name="k_lo", tag="k_lo")
            nc.vector.tensor_copy(out=k_mm, in_=k_sb)
        else:
            q_mm, k_mm = q_sb, k_sb
        # block-diagonal q: one full-contraction matmul computes both heads
        # (the zero blocks annihilate the other head's K rows)
        q_z = const.tile([128, 2 * S], LDT, name="q_z", tag="q_z")
        nc.vector.memset(q_z, 0.0)
        nc.vector.tensor_copy(out=q_z[0:DH, 0:S], in_=q_mm[0:DH, :])
        nc.vector.tensor_copy(out=q_z[DH:2 * DH, S:2 * S], in_=q_mm[DH:2 * DH, :])

        # ---- new-key V (transpose to [t', d] and add ones cols) ----
        vt_ps = mpsum.tile([S, 128], F32, name="vt_ps", tag="mp")
        nc.tensor.transpose(vt_ps, v_sb, ident)
        vaug_new = const.tile([S, 130], VDT, name="vaug_new", tag="vaug_new")
        nc.vector.memset(vaug_new, 1.0)
        nc.scalar.copy(vaug_new[:, 0:64], vt_ps[:, 0:64])
        nc.scalar.copy(vaug_new[:, 65:129], vt_ps[:, 64:128])

        # ---- attention accumulator: both heads in one [64, 130] bank.
        # rows 0:32 x cols 0:65 = head0 [PV | denom]; rows 32:64 x cols
        # 65:130 = head1; other quadrants are unused byproducts ----
        out_acc = opsum.tile([2 * S, 130], F32, name="out_acc", tag="out_acc")

        # new-key block: per-head scores -> exp into a shared [32, 64]
        # stack -> mask -> ONE combined PV matmul (starts accumulation)
        pn = const.tile([S, 2 * S], VDT, name="pn", tag="pn")
        sn = mpsum.tile([S, 2 * S], F32, name="sn", tag="mp")
        nc.tensor.matmul(sn, lhsT=k_mm, rhs=q_z, start=True, stop=True)
        nc.scalar.activation(pn, sn, EXP)
        nc.vector.tensor_mul(out=pn, in0=pn, in1=mask_sb)
        nc.tensor.matmul(out_acc, lhsT=pn, rhs=vaug_new, start=True, stop=False)

        # ---- main loop ----
        GPS2 = 512 // S
        NSG2 = CHUNK // (GPS2 * 128)
        for ci in range(NCHUNK):
            kt_sb = kpool.tile([128, CHUNK], KDT, name="kt_sb", tag="kt")
            nc.sync.dma_start(kt_sb, kt[:, ci * CHUNK:(ci + 1) * CHUNK])
            vc_sb = vpool.tile([128, CHUNK // 128, 130], VDT, name="vc_sb", tag="v")
            nc.sync.dma_start(
                vc_sb,
                vaug[ci * CHUNK:(ci + 1) * CHUNK, :].rearrange("(p g) f -> p g f", p=128))
            GSG = 512 // (2 * S)   # 128-key groups per PSUM bank (8)
            NSG3 = CHUNK // (GSG * 128)
            for sg in range(NSG3):
                sps = spsum.tile([128, 512], F32, name="sps", tag="sc")
                for g in range(GSG):
                    off = sg * GSG * 128 + g * 128
                    nc.tensor.matmul(sps[:, g * 2 * S:(g + 1) * 2 * S],
                                     lhsT=kt_sb[:, off:off + 128],
                                     rhs=q_z, start=True, stop=True)
                pt = ppool.tile([128, 512], VDT, name="pt", tag="pt")
                nc.scalar.activation(pt, sps, EXP)
                last_sg = (ci == NCHUNK - 1) and (sg == NSG3 - 1)
                for g in range(GSG):
                    gi = sg * GSG + g
                    nc.tensor.matmul(out_acc,
                                     lhsT=pt[:, g * 2 * S:(g + 1) * 2 * S],
                                     rhs=vc_sb[:, gi, :],
                                     start=False, stop=(last_sg and g == GSG - 1))

        # ---- epilogue: accumulator -> SBUF, divide, merge, c_proj ----
        acc_sb = const.tile([2 * S, 130], F32, name="acc_sb", tag="acc_sb")
        nc.scalar.copy(acc_sb, out_acc)
        recip = const.tile([2 * S, 1], F32, name="recip", tag="recip")
        nc.vector.reciprocal(recip[0:S], acc_sb[0:S, 64:65])
        nc.vector.reciprocal(recip[S:2 * S], acc_sb[S:2 * S, 129:130])
        a_stack = const.tile([2 * S, 128], F32, name="a_stack", tag="a_stack")
        nc.vector.memset(a_stack, 0.0)
        nc.vector.tensor_scalar_mul(a_stack[0:S, 0:DH],
                                    acc_sb[0:S, 0:DH], recip[0:S])
        nc.vector.tensor_scalar_mul(a_stack[S:2 * S, DH:2 * DH],
                                    acc_sb[S:2 * S, 65:65 + DH], recip[S:2 * S])
        at_ps = mpsum.tile([128, 2 * S], F32, name="at_ps", tag="mp")
        nc.tensor.transpose(at_ps, a_stack, ident[0:2 * S, 0:2 * S])
        at_sb = const.tile([128, 2 * S], F32, name="at_sb", tag="at_sb")
        nc.scalar.copy(at_sb, at_ps)
        out_sb = const.tile([S, N_EMBD], F32, name="out_sb", tag="out_sb")
        for half in range(2):
            pp = mpsum.tile([S, 512], F32, name=f"proj{half}", tag="mp")
            for h in range(HPC):
                nc.tensor.matmul(pp, lhsT=at_sb[:, h * S:(h + 1) * S],
                                 rhs=wp_sb[:, half * 512:(half + 1) * 512],
                                 start=(h == 0), stop=(h == 1))
            nc.scalar.copy(out_sb[:, half * 512:(half + 1) * 512], pp)
        nc.sync.dma_start(po, out_sb)

    nc.compile()
    _CACHE[key] = nc
    return nc


def _np_low(a):
    if LOW_DT == "fp16":
        return np.asarray(a, np.float32).astype(np.float16)
    import ml_dtypes
    return np.asarray(a, np.float32).astype(ml_dtypes.bfloat16)


def make_in_maps(x, layer_past, w_attn, b_attn, w_proj):
    """Host-side sharding: per-core input dict."""
    x = np.ascontiguousarray(np.asarray(x, np.float32))
    layer_past = np.asarray(layer_past, np.float32)
    w_attn = np.asarray(w_attn, np.float32)
    b_attn = np.asarray(b_attn, np.float32)

    xT = np.ascontiguousarray(x.T)                      # [1024, 32]
    mask = (np.arange(S)[:, None] <= np.arange(S)[None, :]).astype(np.float32)
    mask = np.concatenate([mask, mask], axis=1)          # [32, 64] both heads
    in_maps = []
    for c in range(NCORES):
        h0 = HPC * c
        f0 = h0 * DH
        kp = layer_past[0, h0:h0 + HPC]                 # [2, 32768, 64]
        ktc = np.ascontiguousarray(
            kp.transpose(0, 2, 1).reshape(DPC, PAST))   # [128, 32768]
        vp = layer_past[1, h0:h0 + HPC]                 # [2, 32768, 64]
        va = np.ones((PAST, 130), np.float32)
        va[:, 0:64] = vp[0]
        va[:, 65:129] = vp[1]
        # permute rows so each partition's 32 rows per 4096-chunk are contiguous
        va = np.ascontiguousarray(
            va.reshape(PAST // CHUNK, CHUNK // 128, 128, 130)
              .transpose(0, 2, 1, 3).reshape(PAST, 130))
        wqc = np.ascontiguousarray(w_attn[:, f0:f0 + DPC]) / 8.0
        wkc = np.ascontiguousarray(w_attn[:, N_EMBD + f0:N_EMBD + f0 + DPC])
        wvc = np.ascontiguousarray(w_attn[:, 2 * N_EMBD + f0:2 * N_EMBD + f0 + DPC])
        bq = b_attn[f0:f0 + DPC] / 8.0
        bk = b_attn[N_EMBD + f0:N_EMBD + f0 + DPC]
        bv = b_attn[2 * N_EMBD + f0:2 * N_EMBD + f0 + DPC]
        bqkvc = np.ascontiguousarray(
            np.stack([bq, bk, bv], axis=1).astype(np.float32))
        wpc = np.ascontiguousarray(np.asarray(w_proj, np.float32)[f0:f0 + DPC])
        wqc = wqc.astype(np.float32)
        in_maps.append(dict(
            kt=_np_low(ktc) if K_LOW else ktc,
            vaug=_np_low(va) if V_LOW else va,
            xT=_np_low(xT) if QKV_LOW else xT,
            wq=_np_low(wqc) if QKV_LOW else wqc,
            wk=_np_low(wkc) if QKV_LOW else wkc,
            wv=_np_low(wvc) if QKV_LOW else wvc,
            bqkv=bqkvc, wp=wpc,
            maskT=_np_low(mask) if V_LOW else mask))
    return in_maps


def gather(results, x, layer_past, w_attn, b_attn, b_proj):
    """Host-side unshard: sum partials; assemble present from exact f32
    host-side qkv (the new k/v are 0.1% of the FLOPs but half the norm of
    present, so they are kept full-precision)."""
    x = np.asarray(x, np.float32)
    layer_past = np.asarray(layer_past, np.float32)
    w_attn = np.asarray(w_attn, np.float32)
    b_attn = np.asarray(b_attn, np.float32)
    b_proj = np.asarray(b_proj, np.float32)
    out = np.zeros((S, N_EMBD), np.float32)
    for c in range(NCORES):
        out += results[c]["po"]
    out = out + b_proj
    kv = x @ w_attn[:, N_EMBD:] + b_attn[N_EMBD:]      # [32, 2048]
    k_new = kv[:, :N_EMBD].reshape(S, N_HEAD, DH).transpose(1, 0, 2)
    v_new = kv[:, N_EMBD:].reshape(S, N_HEAD, DH).transpose(1, 0, 2)
    present = np.concatenate(
        [layer_past, np.stack([k_new, v_new], axis=0).astype(np.float32)],
        axis=2)
    return out.astype(np.float32), present.astype(np.float32)


def run(inputs, trace=False, **kw):
    """Build (cached), run on 8 cores, return (BassKernelResults, in_maps)."""
    nc = _build()
    in_maps = make_in_maps(inputs["x"], inputs["layer_past"], inputs["w_attn"],
                           inputs["b_attn"], inputs["w_proj"])
    res = run_bass_kernel_spmd(nc, in_maps, core_ids=list(range(NCORES)),
                               trace=trace, **kw)
    return res, in_maps


def kernel(x, layer_past, w_attn, b_attn, w_proj, b_proj, seq_len, past_len):
    assert int(seq_len) == S and int(past_len) == PAST
    inputs = dict(x=x, layer_past=layer_past, w_attn=w_attn, b_attn=b_attn,
                  w_proj=w_proj)
    res, _ = run(inputs)
    return gather(res.results, x, layer_past, w_attn, b_attn, b_proj)


# revision 21
# speedup vs baseline: 1.1559x; 1.1559x over previous
"""Trainium2 Bass kernel: decode attention with a 32K KV cache.

Problem: x[32,1024] -> qkv proj (16 heads, dh=64) -> attention over
(32768 cached + 32 new) keys -> c_proj. Returns (out[32,1024],
present[2,16,32800,64]).

Sharding: 2 heads per core (tensor parallel over n_head=16).
Each core gets:
  kt    [128, 32768]  K^T for its 2 heads (d-major: rows 0:64 head A,
                      64:128 head B) -- stationary operand of the scores
                      matmuls, streamed in 4096-key chunks.
  vaug  [32768, 130]  V rows [v_h0(64) | 1 | v_h1(64) | 1]; the ones
                      columns make the PV matmul also produce the softmax
                      denominator. Rows are host-permuted so the per-chunk
                      DMA is 16.6KB-contiguous per partition.
  xT, wq/wk/wv (per-core columns of w_attn; wq pre-scaled by 1/8),
  bqkv, wp (per-core rows of w_proj), maskT (causal mask for the 32 new
  keys).
Each core returns its partial c_proj output po[32,1024] (contribution of
its 2 heads); the host sums the 8 partials and adds b_proj. No on-device
collective needed. knew/vnew [128,32] return the new k/v for `present`.

Device algorithm per head (S^T layout, no max-subtraction -- scores are
q.k/8 with |score| < ~2, exp is safe in fp32; masked new-block entries
are zeroed multiplicatively after exp):
  per 128-key group g: S^T[g] [128,32] = (K^T chunk).T @ qT   (PSUM)
  per 2048 keys: P^T = exp(S^T bank [128,512])                (ACT->SBUF)
  per group: out[32,65] += P^T[g].T @ [V|1][g]                (PSUM accum)
  epilogue: a = out[:, :64] / out[:, 64:65]; partial = a_merged @ wp.
"""

import numpy as np
from contextlib import ExitStack

import concourse.bass as bass
import concourse.tile as tile
from concourse import bacc, mybir
from concourse.bass_utils import run_bass_kernel_spmd
from concourse.masks import make_identity

N_EMBD = 1024
N_HEAD = 16
DH = 64
S = 32
PAST = 32768
NCORES = 8
HPC = N_HEAD // NCORES        # 2 heads per core
DPC = HPC * DH                # 128 dims per core
F32 = mybir.dt.float32
BF16 = mybir.dt.bfloat16
FP16 = mybir.dt.float16
FP8 = mybir.dt.float8e4
KSCALE = 64.0

# tuning knobs
CHUNK = 4096                  # keys per DMA chunk
SGK = 2048                    # keys per PSUM scores bank (16 groups x 128)
KV_BUFS = 3                   # double-buffer the kv chunk tiles
PT_BUFS = 4
K_LOW = True                  # low-precision K (and q) for scores
V_LOW = True                  # low-precision V (and P^T) for PV
QKV_LOW = True                # low-precision qkv-projection weights + xT
LOW_DT = "fp16"               # "fp16" | "bf16"
K_FP8 = True                  # fp8-e4m3 K (x64 scaled; descaled in exp scale)

_CACHE = {}


def _build():
    key = (CHUNK, SGK, KV_BUFS, PT_BUFS, K_LOW, V_LOW, QKV_LOW, LOW_DT, K_FP8)
    if key in _CACHE:
        return _CACHE[key]

    NCHUNK = PAST // CHUNK
    NSG = CHUNK // SGK        # supergroups per chunk
    GPS = SGK // 128          # 128-key groups per supergroup
    LDT = FP16 if LOW_DT == "fp16" else BF16
    KDT = FP8 if K_FP8 else (LDT if K_LOW else F32)
    VDT = LDT if V_LOW else F32
    WDT = LDT if QKV_LOW else F32

    nc = bacc.Bacc("TRN2", target_bir_lowering=False, debug=False,
                   enable_asserts=False, num_devices=NCORES)

    kt = nc.dram_tensor("kt", [DPC, PAST], KDT, kind="ExternalInput").ap()
    vaug = nc.dram_tensor("vaug", [PAST, 130], VDT, kind="ExternalInput").ap()
    xT = nc.dram_tensor("xT", [N_EMBD, S], WDT, kind="ExternalInput").ap()
    wq = nc.dram_tensor("wq", [N_EMBD, DPC], WDT, kind="ExternalInput").ap()
    wk = nc.dram_tensor("wk", [N_EMBD, DPC], WDT, kind="ExternalInput").ap()
    wv = nc.dram_tensor("wv", [N_EMBD, DPC], WDT, kind="ExternalInput").ap()
    bqkv = nc.dram_tensor("bqkv", [DPC, 3], F32, kind="ExternalInput").ap()
    wp = nc.dram_tensor("wp", [DPC, N_EMBD], F32, kind="ExternalInput").ap()
    maskT = nc.dram_tensor("maskT", [S, 2 * S], VDT, kind="ExternalInput").ap()
    po = nc.dram_tensor("po", [S, N_EMBD], F32, kind="ExternalOutput").ap()
    knew = nc.dram_tensor("knew", [DPC, S], F32, kind="ExternalOutput").ap()
    vnew = nc.dram_tensor("vnew", [DPC, S], F32, kind="ExternalOutput").ap()

    EXP = mybir.ActivationFunctionType.Exp

    with tile.TileContext(nc) as tc, ExitStack() as ctx:
        const = ctx.enter_context(tc.tile_pool(name="const", bufs=1))
        kpool = ctx.enter_context(tc.tile_pool(name="kpool", bufs=KV_BUFS))
        vpool = ctx.enter_context(tc.tile_pool(name="vpool", bufs=KV_BUFS))
        ppool = ctx.enter_context(tc.tile_pool(name="ppool", bufs=PT_BUFS))
        spsum = ctx.enter_context(tc.tile_pool(name="spsum", bufs=4, space="PSUM"))
        opsum = ctx.enter_context(tc.tile_pool(name="opsum", bufs=1, space="PSUM"))
        mpsum = ctx.enter_context(tc.tile_pool(name="mpsum", bufs=2, space="PSUM"))

        # ---- constants / weights ----
        xT_sb = const.tile([128, 8, S], WDT, name="xT_sb", tag="xT_sb")
        nc.sync.dma_start(xT_sb, xT.rearrange("(c p) s -> p c s", p=128))
        w_sb = {}
        for nm, w in (("wq", wq), ("wk", wk), ("wv", wv)):
            t = const.tile([128, 8, DPC], WDT, name=f"{nm}_sb", tag=f"{nm}_sb")
            nc.sync.dma_start(t, w.rearrange("(c p) f -> p c f", p=128))
            w_sb[nm] = t
        b_sb = const.tile([DPC, 3], F32, name="b_sb", tag="b_sb")
        nc.sync.dma_start(b_sb, bqkv)
        mask_sb = const.tile([S, 2 * S], VDT, name="mask_sb", tag="mask_sb")
        nc.sync.dma_start(mask_sb, maskT)
        wp_sb = const.tile([DPC, N_EMBD], F32, name="wp_sb", tag="wp_sb")
        nc.gpsimd.dma_start(wp_sb, wp)
        ident = const.tile([128, 128], F32, name="ident", tag="ident")
        make_identity(nc, ident)

        # ---- qkv projection (transposed layout [f, s]) ----
        qkv = {}
        for bi, nm in enumerate(("wq", "wk", "wv")):
            ps = mpsum.tile([128, S], F32, name=f"qkvp_{nm}", tag="mp")
            for c in range(8):
                nc.tensor.matmul(ps, lhsT=w_sb[nm][:, c, :], rhs=xT_sb[:, c, :],
                                 start=(c == 0), stop=(c == 7))
            sb = const.tile([128, S], F32, name=f"{nm}t", tag=f"{nm}t")
            nc.vector.tensor_scalar_add(sb, ps, b_sb[:, bi:bi + 1])
            qkv[nm] = sb
        q_sb, k_sb, v_sb = qkv["wq"], qkv["wk"], qkv["wv"]

        nc.gpsimd.dma_start(knew, k_sb)
        nc.gpsimd.dma_start(vnew, v_sb)

        if K_LOW:
            q_mm = const.tile([128, S], LDT, name="q_lo", tag="q_lo")
            nc.vector.tensor_copy(out=q_mm, in_=q_sb)
            k_mm = const.tile([128, S], LDT, name="k_lo", tag="k_lo")
            nc.vector.tensor_copy(out=k_mm, in_=k_sb)
        else:
            q_mm, k_mm = q_sb, k_sb
        # block-diagonal q: one full-contraction matmul computes both heads
        # (the zero blocks annihilate the other head's K rows)
        QZDT = FP8 if K_FP8 else LDT
        q_z = const.tile([128, 2 * S], QZDT, name="q_z", tag="q_z")
        nc.vector.memset(q_z, 0.0)
        nc.vector.tensor_copy(out=q_z[0:DH, 0:S], in_=q_mm[0:DH, :])
        nc.vector.tensor_copy(out=q_z[DH:2 * DH, S:2 * S], in_=q_mm[DH:2 * DH, :])
        if K_FP8:
            q_z16 = const.tile([128, 2 * S], LDT, name="q_z16", tag="q_z16")
            nc.vector.memset(q_z16, 0.0)
            nc.vector.tensor_copy(out=q_z16[0:DH, 0:S], in_=q_mm[0:DH, :])
            nc.vector.tensor_copy(out=q_z16[DH:2 * DH, S:2 * S],
                                  in_=q_mm[DH:2 * DH, :])
        else:
            q_z16 = q_z

        # ---- new-key V (transpose to [t', d] and add ones cols) ----
        vt_ps = mpsum.tile([S, 128], F32, name="vt_ps", tag="mp")
        nc.tensor.transpose(vt_ps, v_sb, ident)
        vaug_new = const.tile([S, 130], VDT, name="vaug_new", tag="vaug_new")
        nc.vector.memset(vaug_new, 1.0)
        nc.scalar.copy(vaug_new[:, 0:64], vt_ps[:, 0:64])
        nc.scalar.copy(vaug_new[:, 65:129], vt_ps[:, 64:128])

        # ---- attention accumulator: both heads in one [64, 130] bank.
        # rows 0:32 x cols 0:65 = head0 [PV | denom]; rows 32:64 x cols
        # 65:130 = head1; other quadrants are unused byproducts ----
        out_acc = opsum.tile([2 * S, 130], F32, name="out_acc", tag="out_acc")

        # new-key block: per-head scores -> exp into a shared [32, 64]
        # stack -> mask -> ONE combined PV matmul (starts accumulation)
        pn = const.tile([S, 2 * S], VDT, name="pn", tag="pn")
        sn = mpsum.tile([S, 2 * S], F32, name="sn", tag="mp")
        nc.tensor.matmul(sn, lhsT=k_mm, rhs=q_z16, start=True, stop=True)
        nc.scalar.activation(pn, sn, EXP)
        nc.vector.tensor_mul(out=pn, in0=pn, in1=mask_sb)
        nc.tensor.matmul(out_acc, lhsT=pn, rhs=vaug_new, start=True, stop=False)

        # ---- main loop ----
        GPS2 = 512 // S
        NSG2 = CHUNK // (GPS2 * 128)
        for ci in range(NCHUNK):
            kt_sb = kpool.tile([128, CHUNK], KDT, name="kt_sb", tag="kt")
            nc.sync.dma_start(kt_sb, kt[:, ci * CHUNK:(ci + 1) * CHUNK])
            vc_sb = vpool.tile([128, CHUNK // 128, 130], VDT, name="vc_sb", tag="v")
            nc.sync.dma_start(
                vc_sb,
                vaug[ci * CHUNK:(ci + 1) * CHUNK, :].rearrange("(p g) f -> p g f", p=128))
            GSG = 512 // (2 * S)   # 128-key groups per PSUM bank (8)
            NSG3 = CHUNK // (GSG * 128)
            for sg in range(NSG3):
                sps = spsum.tile([128, 512], F32, name="sps", tag="sc")
                for g in range(GSG):
                    off = sg * GSG * 128 + g * 128
                    nc.tensor.matmul(sps[:, g * 2 * S:(g + 1) * 2 * S],
                                     lhsT=kt_sb[:, off:off + 128],
                                     rhs=q_z, start=True, stop=True)
                pt = ppool.tile([128, 512], VDT, name="pt", tag="pt")
                nc.scalar.activation(pt, sps, EXP,
                                     scale=(1.0 / KSCALE) if K_FP8 else 1.0)
                last_sg = (ci == NCHUNK - 1) and (sg == NSG3 - 1)
                for g in range(GSG):
                    gi = sg * GSG + g
                    nc.tensor.matmul(out_acc,
                                     lhsT=pt[:, g * 2 * S:(g + 1) * 2 * S],
                                     rhs=vc_sb[:, gi, :],
                                     start=False, stop=(last_sg and g == GSG - 1))

        # ---- epilogue: accumulator -> SBUF, divide, merge, c_proj ----
        acc_sb = const.tile([2 * S, 130], F32, name="acc_sb", tag="acc_sb")
        nc.scalar.copy(acc_sb, out_acc)
        recip = const.tile([2 * S, 1], F32, name="recip", tag="recip")
        nc.vector.reciprocal(recip[0:S], acc_sb[0:S, 64:65])
        nc.vector.reciprocal(recip[S:2 * S], acc_sb[S:2 * S, 129:130])
        a_stack = const.tile([2 * S, 128], F32, name="a_stack", tag="a_stack")
        nc.vector.memset(a_stack, 0.0)
        nc.vector.tensor_scalar_mul(a_stack[0:S, 0:DH],
                                    acc_sb[0:S, 0:DH], recip[0:S])
        nc.vector.tensor_scalar_mul(a_stack[S:2 * S, DH:2 * DH],
                                    acc_sb[S:2 * S, 65:65 + DH], recip[S:2 * S])
        at_ps = mpsum.tile([128, 2 * S], F32, name="at_ps", tag="mp")
        nc.tensor.transpose(at_ps, a_stack, ident[0:2 * S, 0:2 * S])
        at_sb = const.tile([128, 2 * S], F32, name="at_sb", tag="at_sb")
        nc.scalar.copy(at_sb, at_ps)
        out_sb = const.tile([S, N_EMBD], F32, name="out_sb", tag="out_sb")
        for half in range(2):
            pp = mpsum.tile([S, 512], F32, name=f"proj{half}", tag="mp")
            for h in range(HPC):
                nc.tensor.matmul(pp, lhsT=at_sb[:, h * S:(h + 1) * S],
                                 rhs=wp_sb[:, half * 512:(half + 1) * 512],
                                 start=(h == 0), stop=(h == 1))
            nc.scalar.copy(out_sb[:, half * 512:(half + 1) * 512], pp)
        nc.sync.dma_start(po, out_sb)

    nc.compile()
    _CACHE[key] = nc
    return nc


def _np_low(a):
    if LOW_DT == "fp16":
        return np.asarray(a, np.float32).astype(np.float16)
    import ml_dtypes
    return np.asarray(a, np.float32).astype(ml_dtypes.bfloat16)


def make_in_maps(x, layer_past, w_attn, b_attn, w_proj):
    """Host-side sharding: per-core input dict."""
    x = np.ascontiguousarray(np.asarray(x, np.float32))
    layer_past = np.asarray(layer_past, np.float32)
    w_attn = np.asarray(w_attn, np.float32)
    b_attn = np.asarray(b_attn, np.float32)

    xT = np.ascontiguousarray(x.T)                      # [1024, 32]
    mask = (np.arange(S)[:, None] <= np.arange(S)[None, :]).astype(np.float32)
    mask = np.concatenate([mask, mask], axis=1)          # [32, 64] both heads
    in_maps = []
    for c in range(NCORES):
        h0 = HPC * c
        f0 = h0 * DH
        kp = layer_past[0, h0:h0 + HPC]                 # [2, 32768, 64]
        ktc = np.ascontiguousarray(
            kp.transpose(0, 2, 1).reshape(DPC, PAST))   # [128, 32768]
        vp = layer_past[1, h0:h0 + HPC]                 # [2, 32768, 64]
        va = np.ones((PAST, 130), np.float32)
        va[:, 0:64] = vp[0]
        va[:, 65:129] = vp[1]
        # permute rows so each partition's 32 rows per 4096-chunk are contiguous
        va = np.ascontiguousarray(
            va.reshape(PAST // CHUNK, CHUNK // 128, 128, 130)
              .transpose(0, 2, 1, 3).reshape(PAST, 130))
        wqc = np.ascontiguousarray(w_attn[:, f0:f0 + DPC]) / 8.0
        wkc = np.ascontiguousarray(w_attn[:, N_EMBD + f0:N_EMBD + f0 + DPC])
        wvc = np.ascontiguousarray(w_attn[:, 2 * N_EMBD + f0:2 * N_EMBD + f0 + DPC])
        bq = b_attn[f0:f0 + DPC] / 8.0
        bk = b_attn[N_EMBD + f0:N_EMBD + f0 + DPC]
        bv = b_attn[2 * N_EMBD + f0:2 * N_EMBD + f0 + DPC]
        bqkvc = np.ascontiguousarray(
            np.stack([bq, bk, bv], axis=1).astype(np.float32))
        wpc = np.ascontiguousarray(np.asarray(w_proj, np.float32)[f0:f0 + DPC])
        wqc = wqc.astype(np.float32)
        if K_FP8:
            import ml_dtypes
            ktx = (ktc * KSCALE).astype(ml_dtypes.float8_e4m3)
        else:
            ktx = _np_low(ktc) if K_LOW else ktc
        in_maps.append(dict(
            kt=ktx,
            vaug=_np_low(va) if V_LOW else va,
            xT=_np_low(xT) if QKV_LOW else xT,
            wq=_np_low(wqc) if QKV_LOW else wqc,
            wk=_np_low(wkc) if QKV_LOW else wkc,
            wv=_np_low(wvc) if QKV_LOW else wvc,
            bqkv=bqkvc, wp=wpc,
            maskT=_np_low(mask) if V_LOW else mask))
    return in_maps


def gather(results, x, layer_past, w_attn, b_attn, b_proj):
    """Host-side unshard: sum partials; assemble present from exact f32
    host-side qkv (the new k/v are 0.1% of the FLOPs but half the norm of
    present, so they are kept full-precision)."""
    x = np.asarray(x, np.float32)
    layer_past = np.asarray(layer_past, np.float32)
    w_attn = np.asarray(w_attn, np.float32)
    b_attn = np.asarray(b_attn, np.float32)
    b_proj = np.asarray(b_proj, np.float32)
    out = np.zeros((S, N_EMBD), np.float32)
    for c in range(NCORES):
        out += results[c]["po"]
    out = out + b_proj
    kv = x @ w_attn[:, N_EMBD:] + b_attn[N_EMBD:]      # [32, 2048]
    k_new = kv[:, :N_EMBD].reshape(S, N_HEAD, DH).transpose(1, 0, 2)
    v_new = kv[:, N_EMBD:].reshape(S, N_HEAD, DH).transpose(1, 0, 2)
    present = np.concatenate(
        [layer_past, np.stack([k_new, v_new], axis=0).astype(np.float32)],
        axis=2)
    return out.astype(np.float32), present.astype(np.float32)


def run(inputs, trace=False, **kw):
    """Build (cached), run on 8 cores, return (BassKernelResults, in_maps)."""
    nc = _build()
    in_maps = make_in_maps(inputs["x"], inputs["layer_past"], inputs["w_attn"],
                           inputs["b_attn"], inputs["w_proj"])
    res = run_bass_kernel_spmd(nc, in_maps, core_ids=list(range(NCORES)),
                               trace=trace, **kw)
    return res, in_maps


def kernel(x, layer_past, w_attn, b_attn, w_proj, b_proj, seq_len, past_len):
    assert int(seq_len) == S and int(past_len) == PAST
    inputs = dict(x=x, layer_past=layer_past, w_attn=w_attn, b_attn=b_attn,
                  w_proj=w_proj)
    res, _ = run(inputs)
    return gather(res.results, x, layer_past, w_attn, b_attn, b_proj)


# revision 22
# speedup vs baseline: 1.2268x; 1.0613x over previous
"""Trainium2 Bass kernel: decode attention with a 32K KV cache.

Problem: x[32,1024] -> qkv proj (16 heads, dh=64) -> attention over
(32768 cached + 32 new) keys -> c_proj. Returns (out[32,1024],
present[2,16,32800,64]).

Sharding: 2 heads per core (tensor parallel over n_head=16).
Each core gets:
  kt    [128, 32768]  K^T for its 2 heads (d-major: rows 0:64 head A,
                      64:128 head B) -- stationary operand of the scores
                      matmuls, streamed in 4096-key chunks.
  vaug  [32768, 130]  V rows [v_h0(64) | 1 | v_h1(64) | 1]; the ones
                      columns make the PV matmul also produce the softmax
                      denominator. Rows are host-permuted so the per-chunk
                      DMA is 16.6KB-contiguous per partition.
  xT, wq/wk/wv (per-core columns of w_attn; wq pre-scaled by 1/8),
  bqkv, wp (per-core rows of w_proj), maskT (causal mask for the 32 new
  keys).
Each core returns its partial c_proj output po[32,1024] (contribution of
its 2 heads); the host sums the 8 partials and adds b_proj. No on-device
collective needed. knew/vnew [128,32] return the new k/v for `present`.

Device algorithm per head (S^T layout, no max-subtraction -- scores are
q.k/8 with |score| < ~2, exp is safe in fp32; masked new-block entries
are zeroed multiplicatively after exp):
  per 128-key group g: S^T[g] [128,32] = (K^T chunk).T @ qT   (PSUM)
  per 2048 keys: P^T = exp(S^T bank [128,512])                (ACT->SBUF)
  per group: out[32,65] += P^T[g].T @ [V|1][g]                (PSUM accum)
  epilogue: a = out[:, :64] / out[:, 64:65]; partial = a_merged @ wp.
"""

import numpy as np
from contextlib import ExitStack

import concourse.bass as bass
import concourse.tile as tile
from concourse import bacc, mybir
from concourse.bass_utils import run_bass_kernel_spmd
from concourse.masks import make_identity

N_EMBD = 1024
N_HEAD = 16
DH = 64
S = 32
PAST = 32768
NCORES = 8
HPC = N_HEAD // NCORES        # 2 heads per core
DPC = HPC * DH                # 128 dims per core
F32 = mybir.dt.float32
BF16 = mybir.dt.bfloat16
FP16 = mybir.dt.float16
FP8 = mybir.dt.float8e4
KSCALE = 64.0

# tuning knobs
CHUNK = 4096                  # keys per DMA chunk
SGK = 2048                    # keys per PSUM scores bank (16 groups x 128)
KV_BUFS = 3                   # double-buffer the kv chunk tiles
PT_BUFS = 6
K_LOW = True                  # low-precision K (and q) for scores
V_LOW = True                  # low-precision V (and P^T) for PV
QKV_LOW = True                # low-precision qkv-projection weights + xT
LOW_DT = "fp16"               # "fp16" | "bf16"
K_FP8 = True                  # fp8-e4m3 K (x64 scaled; descaled in exp scale)

_CACHE = {}


def _build():
    key = (CHUNK, SGK, KV_BUFS, PT_BUFS, K_LOW, V_LOW, QKV_LOW, LOW_DT, K_FP8)
    if key in _CACHE:
        return _CACHE[key]

    NCHUNK = PAST // CHUNK
    NSG = CHUNK // SGK        # supergroups per chunk
    GPS = SGK // 128          # 128-key groups per supergroup
    LDT = FP16 if LOW_DT == "fp16" else BF16
    KDT = FP8 if K_FP8 else (LDT if K_LOW else F32)
    VDT = LDT if V_LOW else F32
    WDT = LDT if QKV_LOW else F32

    nc = bacc.Bacc("TRN2", target_bir_lowering=False, debug=False,
                   enable_asserts=False, num_devices=NCORES)

    kt = nc.dram_tensor("kt", [DPC, PAST], KDT, kind="ExternalInput").ap()
    vaug = nc.dram_tensor("vaug", [PAST, 130], VDT, kind="ExternalInput").ap()
    xT = nc.dram_tensor("xT", [N_EMBD, S], WDT, kind="ExternalInput").ap()
    wq = nc.dram_tensor("wq", [N_EMBD, DPC], WDT, kind="ExternalInput").ap()
    wk = nc.dram_tensor("wk", [N_EMBD, DPC], WDT, kind="ExternalInput").ap()
    wv = nc.dram_tensor("wv", [N_EMBD, DPC], WDT, kind="ExternalInput").ap()
    bqkv = nc.dram_tensor("bqkv", [DPC, 3], F32, kind="ExternalInput").ap()
    wp = nc.dram_tensor("wp", [DPC, N_EMBD], F32, kind="ExternalInput").ap()
    maskT = nc.dram_tensor("maskT", [S, 2 * S], VDT, kind="ExternalInput").ap()
    po = nc.dram_tensor("po", [S, N_EMBD], F32, kind="ExternalOutput").ap()
    knew = nc.dram_tensor("knew", [DPC, S], F32, kind="ExternalOutput").ap()
    vnew = nc.dram_tensor("vnew", [DPC, S], F32, kind="ExternalOutput").ap()

    EXP = mybir.ActivationFunctionType.Exp

    with tile.TileContext(nc) as tc, ExitStack() as ctx:
        const = ctx.enter_context(tc.tile_pool(name="const", bufs=1))
        kpool = ctx.enter_context(tc.tile_pool(name="kpool", bufs=KV_BUFS))
        vpool = ctx.enter_context(tc.tile_pool(name="vpool", bufs=KV_BUFS))
        ppool = ctx.enter_context(tc.tile_pool(name="ppool", bufs=PT_BUFS))
        spsum = ctx.enter_context(tc.tile_pool(name="spsum", bufs=5, space="PSUM"))
        opsum = ctx.enter_context(tc.tile_pool(name="opsum", bufs=1, space="PSUM"))
        mpsum = ctx.enter_context(tc.tile_pool(name="mpsum", bufs=2, space="PSUM"))

        # ---- constants / weights ----
        xT_sb = const.tile([128, 8, S], WDT, name="xT_sb", tag="xT_sb")
        nc.sync.dma_start(xT_sb, xT.rearrange("(c p) s -> p c s", p=128))
        w_sb = {}
        for nm, w in (("wq", wq), ("wk", wk), ("wv", wv)):
            t = const.tile([128, 8, DPC], WDT, name=f"{nm}_sb", tag=f"{nm}_sb")
            nc.sync.dma_start(t, w.rearrange("(c p) f -> p c f", p=128))
            w_sb[nm] = t
        b_sb = const.tile([DPC, 3], F32, name="b_sb", tag="b_sb")
        nc.sync.dma_start(b_sb, bqkv)
        mask_sb = const.tile([S, 2 * S], VDT, name="mask_sb", tag="mask_sb")
        nc.sync.dma_start(mask_sb, maskT)
        wp_sb = const.tile([DPC, N_EMBD], F32, name="wp_sb", tag="wp_sb")
        nc.gpsimd.dma_start(wp_sb, wp)
        ident = const.tile([128, 128], F32, name="ident", tag="ident")
        make_identity(nc, ident)

        # ---- q projection first (everything else can lag) ----
        def proj_block(bi, nm):
            ps = mpsum.tile([128, S], F32, name=f"qkvp_{nm}", tag="mp")
            for c in range(8):
                nc.tensor.matmul(ps, lhsT=w_sb[nm][:, c, :], rhs=xT_sb[:, c, :],
                                 start=(c == 0), stop=(c == 7))
            sb = const.tile([128, S], F32, name=f"{nm}t", tag=f"{nm}t")
            nc.vector.tensor_scalar_add(sb, ps, b_sb[:, bi:bi + 1])
            return sb

        q_sb = proj_block(0, "wq")
        if K_LOW:
            q_mm = const.tile([128, S], LDT, name="q_lo", tag="q_lo")
            nc.vector.tensor_copy(out=q_mm, in_=q_sb)
        else:
            q_mm = q_sb
        # block-diagonal q: one full-contraction matmul computes both heads
        # (the zero blocks annihilate the other head's K rows)
        QZDT = FP8 if K_FP8 else LDT
        q_z = const.tile([128, 2 * S], QZDT, name="q_z", tag="q_z")
        nc.vector.memset(q_z, 0.0)
        nc.vector.tensor_copy(out=q_z[0:DH, 0:S], in_=q_mm[0:DH, :])
        nc.vector.tensor_copy(out=q_z[DH:2 * DH, S:2 * S], in_=q_mm[DH:2 * DH, :])


        # ---- attention accumulator: both heads in one [64, 130] bank.
        # rows 0:32 x cols 0:65 = head0 [PV | denom]; rows 32:64 x cols
        # 65:130 = head1; other quadrants are unused byproducts.
        # Accumulation: chunk PVs first (chunk0/group0 has start=True);
        # the new-key PV closes the group at the end ----
        out_acc = opsum.tile([2 * S, 130], F32, name="out_acc", tag="out_acc")

        # ---- main loop ----
        GPS2 = 512 // S
        NSG2 = CHUNK // (GPS2 * 128)
        for ci in range(NCHUNK):
            kt_sb = kpool.tile([128, CHUNK], KDT, name="kt_sb", tag="kt")
            nc.sync.dma_start(kt_sb, kt[:, ci * CHUNK:(ci + 1) * CHUNK])
            vc_sb = vpool.tile([128, CHUNK // 128, 130], VDT, name="vc_sb", tag="v")
            nc.sync.dma_start(
                vc_sb,
                vaug[ci * CHUNK:(ci + 1) * CHUNK, :].rearrange("(p g) f -> p g f", p=128))
            GSG = 512 // (2 * S)   # 128-key groups per PSUM bank (8)
            NSG3 = CHUNK // (GSG * 128)
            for sg in range(NSG3):
                sps = spsum.tile([128, 512], F32, name="sps", tag="sc")
                for g in range(GSG):
                    off = sg * GSG * 128 + g * 128
                    nc.tensor.matmul(sps[:, g * 2 * S:(g + 1) * 2 * S],
                                     lhsT=kt_sb[:, off:off + 128],
                                     rhs=q_z, start=True, stop=True)
                pt = ppool.tile([128, 512], VDT, name="pt", tag="pt")
                nc.scalar.activation(pt, sps, EXP,
                                     scale=(1.0 / KSCALE) if K_FP8 else 1.0)
                for g in range(GSG):
                    gi = sg * GSG + g
                    first = (ci == 0) and (sg == 0) and (g == 0)
                    nc.tensor.matmul(out_acc,
                                     lhsT=pt[:, g * 2 * S:(g + 1) * 2 * S],
                                     rhs=vc_sb[:, gi, :],
                                     start=first, stop=False)

        # ---- deferred: k/v projection + new-key attention block ----
        k_sb = proj_block(1, "wk")
        v_sb = proj_block(2, "wv")
        nc.gpsimd.dma_start(knew, k_sb)
        nc.gpsimd.dma_start(vnew, v_sb)
        if K_LOW:
            k_mm = const.tile([128, S], LDT, name="k_lo", tag="k_lo")
            nc.vector.tensor_copy(out=k_mm, in_=k_sb)
        else:
            k_mm = k_sb
        q_z16 = const.tile([128, 2 * S], LDT, name="q_z16", tag="q_z16")
        nc.vector.memset(q_z16, 0.0)
        nc.vector.tensor_copy(out=q_z16[0:DH, 0:S], in_=q_mm[0:DH, :])
        nc.vector.tensor_copy(out=q_z16[DH:2 * DH, S:2 * S], in_=q_mm[DH:2 * DH, :])
        vt_ps = mpsum.tile([S, 128], F32, name="vt_ps", tag="mp")
        nc.tensor.transpose(vt_ps, v_sb, ident)
        vaug_new = const.tile([S, 130], VDT, name="vaug_new", tag="vaug_new")
        nc.vector.memset(vaug_new, 1.0)
        nc.scalar.copy(vaug_new[:, 0:64], vt_ps[:, 0:64])
        nc.scalar.copy(vaug_new[:, 65:129], vt_ps[:, 64:128])
        pn = const.tile([S, 2 * S], VDT, name="pn", tag="pn")
        sn = mpsum.tile([S, 2 * S], F32, name="sn", tag="mp")
        nc.tensor.matmul(sn, lhsT=k_mm, rhs=q_z16, start=True, stop=True)
        nc.scalar.activation(pn, sn, EXP)
        nc.vector.tensor_mul(out=pn, in0=pn, in1=mask_sb)
        nc.tensor.matmul(out_acc, lhsT=pn, rhs=vaug_new, start=False, stop=True)

        # ---- epilogue: accumulator -> SBUF, divide, merge, c_proj ----
        acc_sb = const.tile([2 * S, 130], F32, name="acc_sb", tag="acc_sb")
        nc.scalar.copy(acc_sb, out_acc)
        recip = const.tile([2 * S, 1], F32, name="recip", tag="recip")
        nc.vector.reciprocal(recip[0:S], acc_sb[0:S, 64:65])
        nc.vector.reciprocal(recip[S:2 * S], acc_sb[S:2 * S, 129:130])
        a_stack = const.tile([2 * S, 128], F32, name="a_stack", tag="a_stack")
        nc.vector.memset(a_stack, 0.0)
        nc.vector.tensor_scalar_mul(a_stack[0:S, 0:DH],
                                    acc_sb[0:S, 0:DH], recip[0:S])
        nc.vector.tensor_scalar_mul(a_stack[S:2 * S, DH:2 * DH],
                                    acc_sb[S:2 * S, 65:65 + DH], recip[S:2 * S])
        at_ps = mpsum.tile([128, 2 * S], F32, name="at_ps", tag="mp")
        nc.tensor.transpose(at_ps, a_stack, ident[0:2 * S, 0:2 * S])
        at_sb = const.tile([128, 2 * S], F32, name="at_sb", tag="at_sb")
        nc.scalar.copy(at_sb, at_ps)
        out_sb = const.tile([S, N_EMBD], F32, name="out_sb", tag="out_sb")
        for half in range(2):
            pp = mpsum.tile([S, 512], F32, name=f"proj{half}", tag="mp")
            for h in range(HPC):
                nc.tensor.matmul(pp, lhsT=at_sb[:, h * S:(h + 1) * S],
                                 rhs=wp_sb[:, half * 512:(half + 1) * 512],
                                 start=(h == 0), stop=(h == 1))
            nc.scalar.copy(out_sb[:, half * 512:(half + 1) * 512], pp)
        nc.sync.dma_start(po, out_sb)

    nc.compile()
    _CACHE[key] = nc
    return nc


def _np_low(a):
    if LOW_DT == "fp16":
        return np.asarray(a, np.float32).astype(np.float16)
    import ml_dtypes
    return np.asarray(a, np.float32).astype(ml_dtypes.bfloat16)


def make_in_maps(x, layer_past, w_attn, b_attn, w_proj):
    """Host-side sharding: per-core input dict."""
    x = np.ascontiguousarray(np.asarray(x, np.float32))
    layer_past = np.asarray(layer_past, np.float32)
    w_attn = np.asarray(w_attn, np.float32)
    b_attn = np.asarray(b_attn, np.float32)

    xT = np.ascontiguousarray(x.T)                      # [1024, 32]
    mask = (np.arange(S)[:, None] <= np.arange(S)[None, :]).astype(np.float32)
    mask = np.concatenate([mask, mask], axis=1)          # [32, 64] both heads
    in_maps = []
    for c in range(NCORES):
        h0 = HPC * c
        f0 = h0 * DH
        kp = layer_past[0, h0:h0 + HPC]                 # [2, 32768, 64]
        ktc = np.ascontiguousarray(
            kp.transpose(0, 2, 1).reshape(DPC, PAST))   # [128, 32768]
        vp = layer_past[1, h0:h0 + HPC]                 # [2, 32768, 64]
        va = np.ones((PAST, 130), np.float32)
        va[:, 0:64] = vp[0]
        va[:, 65:129] = vp[1]
        # permute rows so each partition's 32 rows per 4096-chunk are contiguous
        va = np.ascontiguousarray(
            va.reshape(PAST // CHUNK, CHUNK // 128, 128, 130)
              .transpose(0, 2, 1, 3).reshape(PAST, 130))
        wqc = np.ascontiguousarray(w_attn[:, f0:f0 + DPC]) / 8.0
        wkc = np.ascontiguousarray(w_attn[:, N_EMBD + f0:N_EMBD + f0 + DPC])
        wvc = np.ascontiguousarray(w_attn[:, 2 * N_EMBD + f0:2 * N_EMBD + f0 + DPC])
        bq = b_attn[f0:f0 + DPC] / 8.0
        bk = b_attn[N_EMBD + f0:N_EMBD + f0 + DPC]
        bv = b_attn[2 * N_EMBD + f0:2 * N_EMBD + f0 + DPC]
        bqkvc = np.ascontiguousarray(
            np.stack([bq, bk, bv], axis=1).astype(np.float32))
        wpc = np.ascontiguousarray(np.asarray(w_proj, np.float32)[f0:f0 + DPC])
        wqc = wqc.astype(np.float32)
        if K_FP8:
            import ml_dtypes
            ktx = (ktc * KSCALE).astype(ml_dtypes.float8_e4m3)
        else:
            ktx = _np_low(ktc) if K_LOW else ktc
        in_maps.append(dict(
            kt=ktx,
            vaug=_np_low(va) if V_LOW else va,
            xT=_np_low(xT) if QKV_LOW else xT,
            wq=_np_low(wqc) if QKV_LOW else wqc,
            wk=_np_low(wkc) if QKV_LOW else wkc,
            wv=_np_low(wvc) if QKV_LOW else wvc,
            bqkv=bqkvc, wp=wpc,
            maskT=_np_low(mask) if V_LOW else mask))
    return in_maps


def gather(results, x, layer_past, w_attn, b_attn, b_proj):
    """Host-side unshard: sum partials; assemble present from exact f32
    host-side qkv (the new k/v are 0.1% of the FLOPs but half the norm of
    present, so they are kept full-precision)."""
    x = np.asarray(x, np.float32)
    layer_past = np.asarray(layer_past, np.float32)
    w_attn = np.asarray(w_attn, np.float32)
    b_attn = np.asarray(b_attn, np.float32)
    b_proj = np.asarray(b_proj, np.float32)
    out = np.zeros((S, N_EMBD), np.float32)
    for c in range(NCORES):
        out += results[c]["po"]
    out = out + b_proj
    kv = x @ w_attn[:, N_EMBD:] + b_attn[N_EMBD:]      # [32, 2048]
    k_new = kv[:, :N_EMBD].reshape(S, N_HEAD, DH).transpose(1, 0, 2)
    v_new = kv[:, N_EMBD:].reshape(S, N_HEAD, DH).transpose(1, 0, 2)
    present = np.concatenate(
        [layer_past, np.stack([k_new, v_new], axis=0).astype(np.float32)],
        axis=2)
    return out.astype(np.float32), present.astype(np.float32)


def run(inputs, trace=False, **kw):
    """Build (cached), run on 8 cores, return (BassKernelResults, in_maps)."""
    nc = _build()
    in_maps = make_in_maps(inputs["x"], inputs["layer_past"], inputs["w_attn"],
                           inputs["b_attn"], inputs["w_proj"])
    res = run_bass_kernel_spmd(nc, in_maps, core_ids=list(range(NCORES)),
                               trace=trace, **kw)
    return res, in_maps


def kernel(x, layer_past, w_attn, b_attn, w_proj, b_proj, seq_len, past_len):
    assert int(seq_len) == S and int(past_len) == PAST
    inputs = dict(x=x, layer_past=layer_past, w_attn=w_attn, b_attn=b_attn,
                  w_proj=w_proj)
    res, _ = run(inputs)
    return gather(res.results, x, layer_past, w_attn, b_attn, b_proj)
